# revision 33
# baseline (speedup 1.0000x reference)
# Trainium2 Bass kernel for nn_CrossAttention_noise (B=4, T1=T2=1024, D=1024,
# H=16, DK=64, K=13, FF=4096), SPMD over 8 NeuronCores.
#
# Sharding: core i handles batch b=i//2 and query-token half t0=(i%2)*512.
# Each core computes its 512 output tokens end-to-end (the K/V convolution
# over the full clean sequence is duplicated between the two cores of a
# batch; no collectives).  Big matmuls run in bf16 with fp32 PSUM
# accumulation; layernorms / softmax / residuals stay fp32.
#
# Layout convention: "T" suffix = channels on partitions, tokens on the free
# dim (the natural matmul layout here); plain tiles = tokens on partitions.
import numpy as np
import ml_dtypes
from contextlib import ExitStack

import concourse.bass as bass
import concourse.mybir as mybir
import concourse.tile as tile
from concourse import bacc
from concourse.bass_utils import run_bass_kernel_spmd
from concourse.masks import make_identity

BF16 = mybir.dt.bfloat16
F32 = mybir.dt.float32
AF = mybir.ActivationFunctionType
ALU = mybir.AluOpType
AX = mybir.AxisListType

B, T, D, H, DK, KW, FF = 4, 1024, 1024, 16, 64, 13, 4096
TQ = 512          # query tokens per core
NHW = 768         # noisy halo window rows (zero-padded on host)
NT2W = 528        # nt2 width (valid cols 0..523)
CT2W = 1040       # ct2 width (valid cols 0..1035)
P = 128
EPS1, EPS2 = 1e-5, 1e-6
NEG = -1.0e30


def _ln_apply(nc, pool, x, out, eps_ap, act_apply=False):
    """out = (x - mean)/sqrt(var + eps) rowwise; x [p, D] f32 in SBUF.

    Heavy passes run on the scalar engine (accum_out reductions + fused
    scale/bias apply); DVE only does tiny [p,1] ops."""
    p = x.shape[0]
    s = pool.tile([P, 1], F32, tag="ln_s", name="ln_s")[:p]
    sq = pool.tile([P, 1], F32, tag="ln_sq", name="ln_sq")[:p]
    scr = pool.tile([P, D], BF16, tag="ln_scr", name="ln_scr", bufs=2)[:p]
    nc.vector.reduce_sum(s, x, axis=AX.X)
    nc.scalar.activation(scr, x, AF.Square, accum_out=sq)
    mu = pool.tile([P, 1], F32, tag="ln_mu", name="ln_mu")[:p]
    nc.vector.tensor_scalar_mul(mu, s, 1.0 / D)
    musq = pool.tile([P, 1], F32, tag="ln_musq", name="ln_musq")[:p]
    nc.vector.tensor_tensor(musq, mu, mu, ALU.mult)
    var = pool.tile([P, 1], F32, tag="ln_var", name="ln_var")[:p]
    nc.vector.tensor_scalar(var, sq, 1.0 / D, musq, ALU.mult, ALU.subtract)
    std = pool.tile([P, 1], F32, tag="ln_std", name="ln_std")[:p]
    nc.scalar.activation(std, var, AF.Sqrt, bias=eps_ap[:p])
    rstd = pool.tile([P, 1], F32, tag="ln_rstd", name="ln_rstd")[:p]
    nc.vector.reciprocal(rstd, std)
    beta = pool.tile([P, 1], F32, tag="ln_beta", name="ln_beta")[:p]
    nc.vector.tensor_tensor(beta, mu, rstd, ALU.mult)
    nc.vector.tensor_scalar_mul(beta, beta, -1.0)
    if act_apply:
        nc.scalar.activation(out, x, AF.Identity, bias=beta, scale=rstd)
    else:
        nc.vector.tensor_scalar(out, x, rstd, beta, ALU.mult, ALU.add)


def _nt2_rng(r, shift, width):
    """Dest/src col ranges for copying transpose block r into a shifted row."""
    lo = r * P - shift
    hi = lo + P
    d0, d1 = max(lo, 0), min(hi, width)
    if d1 <= d0:
        return None
    return d0, d1, d0 - lo


def build_nc():
    nc = bacc.Bacc("TRN2", target_bir_lowering=False, debug=False,
                   num_devices=8)
    dt = {}

    def din(name, shape, dtype):
        dt[name] = nc.dram_tensor(name, list(shape), dtype,
                                  kind="ExternalInput").ap()

    din("noisyH", (NHW, D), F32)          # rows [t0-128, t0+640), zero padded
    din("clean", (T, D), F32)
    din("hm", (P, NHW), BF16)              # halo-token validity (rows equal)
    din("maskb", (T,), F32)               # 0 / -1e30 additive key mask
    din("mod", (6, D), F32)               # sh_msa,1+sc_msa,g_msa,sh_mlp,1+sc_mlp,g_mlp
    din("lng", (P, D), F32)               # ln_noisy_g broadcast to 128 rows
    din("lnb", (P, D), F32)
    din("clng", (D,), F32)
    din("clnb", (D,), F32)
    din("wql", (P, H, 7, DK), BF16)
    din("wkl", (P, H, 7, DK), BF16)
    din("wvl", (P, H, 7, DK), BF16)
    din("bq", (D,), F32)
    din("bk", (D,), F32)
    din("bv", (D,), F32)
    din("fcw", (8, P, 8, P), BF16)        # fc_w.T tiles [mc][kp][ko][mj]
    din("fcb", (D,), F32)
    din("w1t", (32, P, 8, P), BF16)       # ff_w1.T tiles [mc][kp][ko][mj]
    din("fb1", (FF,), F32)
    din("w2t", (8, 4, P, 8, P), BF16)     # ff_w2.T tiles [mc][kq][kp][k8][mj]
    din("fb2", (D,), F32)
    out_ap = nc.dram_tensor("out", [TQ, D], F32, kind="ExternalOutput").ap()

    with tile.TileContext(nc) as tc:
        _emit(tc, dt, out_ap)
    nc.compile()
    return nc


def _emit(tc, dt, out_ap):
    nc = tc.nc
    with ExitStack() as ctx:
        const = ctx.enter_context(tc.tile_pool(name="const", bufs=1))
        small = ctx.enter_context(tc.tile_pool(name="small", bufs=3))
        lnio = ctx.enter_context(tc.tile_pool(name="lnio", bufs=3))
        big = ctx.enter_context(tc.tile_pool(name="bigsb", bufs=1))
        trans = ctx.enter_context(tc.tile_pool(name="trans", bufs=3))
        wpool = ctx.enter_context(tc.tile_pool(name="wstream", bufs=6))
        psc = ctx.enter_context(tc.tile_pool(name="psc", bufs=3, space="PSUM"))
        ppv = ctx.enter_context(tc.tile_pool(name="ppv", bufs=2, space="PSUM"))
        ptp = ctx.enter_context(tc.tile_pool(name="ptp", bufs=1, space="PSUM"))
        psm = ctx.enter_context(tc.tile_pool(name="psm", bufs=2, space="PSUM"))

        ident = const.tile([P, P], BF16)
        make_identity(nc, ident)
        eps1_t = const.tile([P, 1], F32)
        nc.vector.memset(eps1_t, EPS1)
        eps2_t = const.tile([P, 1], F32)
        nc.vector.memset(eps2_t, EPS2)

        def chanvec(name, w=8):
            t = const.tile([P, w], F32, tag=f"cv_{name}")
            nc.sync.dma_start(t, dt[name].rearrange("(m p) -> p m", p=P))
            return t

        bq_s, bk_s, bv_s = chanvec("bq"), chanvec("bk"), chanvec("bv")
        fcb_s, fb2_s = chanvec("fcb"), chanvec("fb2")
        clng_s, clnb_s = chanvec("clng"), chanvec("clnb")
        maskb_s = chanvec("maskb")
        fb1_s = chanvec("fb1", 32)
        mod_s = const.tile([P, 6, 8], F32)
        for s in range(6):
            nc.sync.dma_start(mod_s[:, s, :],
                              dt["mod"][s].rearrange("(m p) -> p m", p=P))
        sh_msa, sc_msa, g_msa = mod_s[:, 0, :], mod_s[:, 1, :], mod_s[:, 2, :]
        sh_mlp, sc_mlp, g_mlp = mod_s[:, 3, :], mod_s[:, 4, :], mod_s[:, 5, :]
        hm_s = const.tile([P, NHW], BF16)
        nc.sync.dma_start(hm_s, dt["hm"])
        lng_s = const.tile([P, D], F32)
        nc.sync.dma_start(lng_s, dt["lng"])
        lnb_s = const.tile([P, D], F32)
        nc.sync.dma_start(lnb_s, dt["lnb"])

        xres = big.tile([P, 4, D], F32)        # LN1 rows [t0, t0+512); later x
        attnT = big.tile([P, 8, TQ], BF16)     # concat_h(out_h/l_h), chan-major

        with tc.tile_pool(name="bigc", bufs=1) as bigc:
            # ---- Phase A: noisy LNs -> nt2 builds -> all q convs ------------
            lnpN_cm = tc.tile_pool(name="lnpN", bufs=1)
            lnpN = lnpN_cm.__enter__()
            lnall = [lnpN.tile([P, D], BF16, name=f"lnall_{i}")
                     for i in range(6)]  # noisy ln2 tiles
            for r in range(6):
                xt = lnio.tile([P, D], F32, tag="ln_in", bufs=2)
                nc.sync.dma_start(xt, dt["noisyH"][r * P:(r + 1) * P, :])
                if 1 <= r <= 4:
                    ln1 = xres[:, r - 1, :]
                else:
                    ln1 = lnio.tile([P, D], F32, tag="ln1_tmp", bufs=1)
                _ln_apply(nc, small, xt, ln1, eps1_t)
                nc.vector.tensor_tensor(ln1, ln1, lng_s, ALU.mult)
                nc.vector.tensor_tensor(ln1, ln1, lnb_s, ALU.add)
                _ln_apply(nc, small, ln1, lnall[r], eps2_t)

            nt2s, ct2s = [], []
            cp_eng = [nc.vector, nc.gpsimd]
            for m in range(8):
                nt2m = bigc.tile([P, 2, NT2W], BF16, name=f"nt2_{m}")
                nt2s.append(nt2m)
                tmn = trans.tile([P, NHW], BF16, tag="tmn", bufs=2)
                for r in range(6):
                    pt = ptp.tile([P, P], BF16, tag="tpbf")
                    nc.tensor.transpose(pt, lnall[r][:, m * P:(m + 1) * P],
                                        ident)
                    dst = tmn[:, r * P:(r + 1) * P]
                    if r % 2 == 0:
                        nc.vector.tensor_scalar(dst, pt, sc_msa[:, m:m + 1],
                                                sh_msa[:, m:m + 1],
                                                ALU.mult, ALU.add)
                    else:
                        nc.scalar.activation(dst, pt, AF.Identity,
                                             bias=sh_msa[:, m:m + 1],
                                             scale=sc_msa[:, m:m + 1])
                for hh in range(2):
                    sl = slice(hh * DK, (hh + 1) * DK)
                    e0, e1 = cp_eng[hh], cp_eng[1 - hh]
                    e0.tensor_tensor(nt2m[0:DK, hh, 0:524], tmn[sl, 122:646],
                                     hm_s[sl, 122:646], ALU.mult)
                    e1.tensor_tensor(nt2m[DK:P, hh, 0:524], tmn[sl, 123:647],
                                     hm_s[sl, 123:647], ALU.mult)

            lnpN_cm.__exit__(None, None, None)
            hpool_cm = tc.tile_pool(name="hpool", bufs=2)
            hpool = hpool_cm.__enter__()

            def conv(h, wname, bias_s, x2, nchunk, name, bufs=2):
                hp, hc = h % 2, h // 2
                wsb = wpool.tile([P, 7, DK], BF16, tag="convw", bufs=4,
                                 name=f"w_{name}")
                nc.sync.dma_start(wsb, dt[wname][:, h])
                outT = hpool.tile([DK, nchunk * TQ], BF16, tag=f"cv_{name}",
                                  bufs=bufs, name=f"cv_{name}_{h}")
                for c in range(nchunk):
                    ps = psm.tile([DK, TQ], F32, tag="conv")
                    for j in range(7):
                        nc.tensor.matmul(
                            ps, wsb[:, j, :],
                            x2[:, hp, c * TQ + 2 * j:c * TQ + 2 * j + TQ],
                            start=(j == 0), stop=(j == 6))
                    if (h + c) % 2 == 0:
                        nc.vector.tensor_scalar_add(
                            outT[:, c * TQ:(c + 1) * TQ], ps,
                            bias_s[hp * DK:(hp + 1) * DK, hc:hc + 1])
                    else:
                        nc.scalar.activation(
                            outT[:, c * TQ:(c + 1) * TQ], ps, AF.Identity,
                            bias=bias_s[hp * DK:(hp + 1) * DK, hc:hc + 1])
                return outT

            qTs = [conv(h, "wql", bq_s, nt2s[h // 2], 1, f"q{h}", bufs=1)
                   for h in range(H)]

            # ---- Phase B: clean LNs -> ct2 builds ---------------------------
            lnpC_cm = tc.tile_pool(name="lnpC", bufs=1)
            lnpC = lnpC_cm.__enter__()
            clnall = [lnpC.tile([P, D], BF16, name=f"clnall_{i}")
                      for i in range(8)]
            for r in range(8):
                xt = lnio.tile([P, D], F32, tag="ln_in", bufs=2)
                nc.sync.dma_start(xt, dt["clean"][r * P:(r + 1) * P, :])
                _ln_apply(nc, small, xt, clnall[r], eps1_t)
            for m in range(8):
                ct2m = bigc.tile([P, 2, CT2W], BF16, name=f"ct2_{m}")
                ct2s.append(ct2m)
                for hh in range(2):
                    nc.gpsimd.memset(ct2m[0:DK, hh, 0:6], 0.0)
                    nc.gpsimd.memset(ct2m[0:DK, hh, 1030:CT2W], 0.0)
                    nc.gpsimd.memset(ct2m[DK:P, hh, 0:5], 0.0)
                    nc.gpsimd.memset(ct2m[DK:P, hh, 1029:CT2W], 0.0)
                tmc = trans.tile([P, T], BF16, tag="tmc", bufs=2)
                for r in range(8):
                    pt = ptp.tile([P, P], BF16, tag="tpbf")
                    nc.tensor.transpose(pt, clnall[r][:, m * P:(m + 1) * P],
                                        ident)
                    dst = tmc[:, r * P:(r + 1) * P]
                    if r % 2 == 0:
                        nc.vector.tensor_scalar(dst, pt, clng_s[:, m:m + 1],
                                                clnb_s[:, m:m + 1],
                                                ALU.mult, ALU.add)
                    else:
                        nc.scalar.activation(dst, pt, AF.Identity,
                                             bias=clnb_s[:, m:m + 1],
                                             scale=clng_s[:, m:m + 1])
                for hh in range(2):
                    sl = slice(hh * DK, (hh + 1) * DK)
                    e0, e1 = cp_eng[hh], cp_eng[1 - hh]
                    e0.tensor_copy(ct2m[0:DK, hh, 6:1030], tmc[sl, :])
                    e1.tensor_copy(ct2m[DK:P, hh, 5:1029], tmc[sl, :])
            lnpC_cm.__exit__(None, None, None)

            # ---- Phase C: per-head conv K/V + cross attention ---------------
            for h in range(H):
                hp = h % 2
                hc = h // 2
                ct2 = ct2s[hc]
                kT = conv(h, "wkl", bk_s, ct2, 2, "k")
                vT = conv(h, "wvl", bv_s, ct2, 2, "v")
                qT = qTs[h]

                # v65: v tokens-on-partitions plus ones column for row sums
                v65 = hpool.tile([P, 8, 66], BF16, tag="v65", bufs=1)
                nc.vector.memset(v65[:, :, 64:65], 1.0)
                for c in range(8):
                    pt = ptp.tile([P, P], BF16, tag="tpbf")
                    nc.tensor.transpose(pt[:, :DK], vT[:, c * P:(c + 1) * P],
                                        ident[:DK, :DK])
                    nc.vector.tensor_copy(v65[:, c, 0:DK], pt[:, :DK])

                # transposed scores; fused mask/scale/exp (T2 on partitions)
                pT = hpool.tile([P, 8, TQ], BF16, tag="pT", bufs=2)
                for c in range(8):
                    ps = psc.tile([P, TQ], F32, tag="sc")
                    nc.tensor.matmul(ps, kT[:, c * P:(c + 1) * P], qT,
                                     start=True, stop=True)
                    nc.scalar.activation(pT[:, c, :], ps, AF.Exp,
                                         bias=maskb_s[:, c:c + 1], scale=0.125)

                # PV: out[65, TQ] = [v|1]^T @ p (row 64 = softmax denominator)
                pv = ppv.tile([P, TQ], F32, tag="pv")
                for c in range(8):
                    nc.tensor.matmul(pv[:65, :], v65[:, c, 0:65], pT[:, c, :],
                                     start=(c == 0), stop=(c == 7))
                linv = trans.tile([1, TQ], F32, tag="linv")
                nc.vector.reciprocal(linv, pv[64:65, :])
                bc_sb = trans.tile([DK, TQ], F32, tag="bcsb", bufs=2)
                nc.gpsimd.partition_broadcast(bc_sb, linv)
                nc.vector.tensor_tensor(attnT[hp * DK:(hp + 1) * DK, hc, :],
                                        pv[0:DK, :], bc_sb, ALU.mult)
            hpool_cm.__exit__(None, None, None)

        # ---- Phase D: fc projection + gate + residual into xres -------------
        fcgs = []
        for m in range(8):
            wt = wpool.tile([P, 8, P], BF16, tag="wt")
            nc.sync.dma_start(wt, dt["fcw"][m])
            ps = psc.tile([P, TQ], F32, tag="sc")
            for k in range(8):
                nc.tensor.matmul(ps, wt[:, k, :], attnT[:, k, :],
                                 start=(k == 0), stop=(k == 7))
            fcg = trans.tile([P, TQ], BF16, tag="fcg", bufs=8,
                             name=f"fcg_{m}")
            nc.vector.tensor_scalar(fcg, ps, fcb_s[:, m:m + 1],
                                    g_msa[:, m:m + 1], ALU.add, ALU.mult)
            fcgs.append(fcg)
        for j in range(4):
            for m in range(8):
                pt = ptp.tile([P, P], BF16, tag="tpbf")
                nc.tensor.transpose(pt, fcgs[m][:, j * P:(j + 1) * P], ident)
                nc.vector.tensor_tensor(xres[:, j, m * P:(m + 1) * P], pt,
                                        xres[:, j, m * P:(m + 1) * P], ALU.add)

        # ---- Phase E: LN3 + mlp modulation -> n2T ---------------------------
        bigf_cm = tc.tile_pool(name="bigf", bufs=1)
        bigf = bigf_cm.__enter__()
        n2T = bigf.tile([P, 8, TQ], BF16)
        for s in range(4):
            l3 = lnio.tile([P, D], BF16, tag="ln2b")
            _ln_apply(nc, small, xres[:, s, :], l3, eps2_t)
            for m in range(8):
                pt = ptp.tile([P, P], BF16, tag="tpbf")
                nc.tensor.transpose(pt, l3[:, m * P:(m + 1) * P], ident)
                nc.vector.tensor_scalar(n2T[:, m, s * P:(s + 1) * P], pt,
                                        sc_mlp[:, m:m + 1], sh_mlp[:, m:m + 1],
                                        ALU.mult, ALU.add)

        # ---- Phase F: FFN (single pass; SBUF freed by bigc/hpool exit) ------
        if True:
            ffa = bigf.tile([P, 32, TQ], BF16)
            for m in range(32):
                wt = wpool.tile([P, 8, P], BF16, tag="wt")
                nc.sync.dma_start(wt, dt["w1t"][m])
                ps = psc.tile([P, TQ], F32, tag="sc")
                for k in range(8):
                    nc.tensor.matmul(ps, wt[:, k, :], n2T[:, k, :],
                                     start=(k == 0), stop=(k == 7))
                nc.scalar.activation(ffa[:, m, :], ps, AF.Gelu_apprx_tanh,
                                     bias=fb1_s[:, m:m + 1])
            for m in range(8):
                ps = psc.tile([P, TQ], F32, tag="sc")
                for kq in range(4):
                    wt = wpool.tile([P, 8, P], BF16, tag="wt")
                    nc.sync.dma_start(wt, dt["w2t"][m, kq])
                    for k8 in range(8):
                        k = kq * 8 + k8
                        nc.tensor.matmul(ps, wt[:, k8, :], ffa[:, k, :],
                                         start=(k == 0), stop=(k == 31))
                ffog = trans.tile([P, TQ], BF16, tag="ffog", bufs=2)
                nc.vector.tensor_scalar(ffog, ps, fb2_s[:, m:m + 1],
                                        g_mlp[:, m:m + 1], ALU.add, ALU.mult)
                for j in range(4):
                    pt = ptp.tile([P, P], BF16, tag="tpbf")
                    nc.tensor.transpose(pt, ffog[:, j * P:(j + 1) * P], ident)
                    nc.vector.tensor_tensor(xres[:, j, m * P:(m + 1) * P], pt,
                                            xres[:, j, m * P:(m + 1) * P],
                                            ALU.add)
        bigf_cm.__exit__(None, None, None)

        for s in range(4):
            nc.sync.dma_start(out_ap[s * P:(s + 1) * P, :], xres[:, s, :])


# --------------------------- host side --------------------------------------
_NC_CACHE = None


def _prep_conv_w(w):
    # w: (D, DK, KW) grouped conv weights -> [128, H, 7, DK] bf16 tap-pair lhsT
    wr = w.reshape(H, DK, DK, KW)                      # [h, m, c, tap]
    arr = np.zeros((P, H, 7, DK), np.float32)
    arr[0:DK] = wr[:, :, :, 0::2].transpose(2, 0, 3, 1)      # taps 0,2,..,12
    arr[DK:P, :, 0:6] = wr[:, :, :, 1::2].transpose(2, 0, 3, 1)
    return arr.astype(ml_dtypes.bfloat16)


def kernel(**inputs):
    global _NC_CACHE
    if _NC_CACHE is None:
        _NC_CACHE = build_nc()
    nc = _NC_CACHE

    f32 = np.float32
    bf = ml_dtypes.bfloat16
    noisy = np.asarray(inputs["noisy_feats"], f32)
    clean = np.asarray(inputs["clean_feats"], f32)
    t = np.asarray(inputs["t"], f32)
    clean_len = np.asarray(inputs["clean_lengths"]).astype(np.int64)

    # AdaLayerNormZero on host (0.02% of FLOPs): emb = silu(t) @ ada_w.T + b
    st = t * (1.0 / (1.0 + np.exp(-t, dtype=f32)))
    emb = st @ np.asarray(inputs["ada_w"], f32).T + np.asarray(inputs["ada_b"], f32)
    sh_msa, sc_msa, g_msa, sh_mlp, sc_mlp, g_mlp = np.split(emb, 6, axis=1)

    wql = _prep_conv_w(np.asarray(inputs["wq"], f32))
    wkl = _prep_conv_w(np.asarray(inputs["wk"], f32))
    wvl = _prep_conv_w(np.asarray(inputs["wv"], f32))
    fcw = np.asarray(inputs["fc_w"], f32).T.reshape(8, P, 8, P) \
        .transpose(2, 1, 0, 3).astype(bf).copy()
    w1t = np.asarray(inputs["ff_w1"], f32).T.reshape(8, P, 32, P) \
        .transpose(2, 1, 0, 3).astype(bf).copy()
    w2t = np.asarray(inputs["ff_w2"], f32).T.reshape(32, P, 8, P) \
        .transpose(2, 0, 1, 3).reshape(8, 4, 8, P, P) \
        .transpose(0, 1, 3, 2, 4).astype(bf).copy()

    common = dict(
        lng=np.broadcast_to(np.asarray(inputs["ln_noisy_g"], f32), (P, D)).copy(),
        lnb=np.broadcast_to(np.asarray(inputs["ln_noisy_b"], f32), (P, D)).copy(),
        clng=np.asarray(inputs["ln_clean_g"], f32).copy(),
        clnb=np.asarray(inputs["ln_clean_b"], f32).copy(),
        wql=wql, wkl=wkl, wvl=wvl,
        bq=np.asarray(inputs["bq"], f32).copy(),
        bk=np.asarray(inputs["bk"], f32).copy(),
        bv=np.asarray(inputs["bv"], f32).copy(),
        fcw=fcw, fcb=np.asarray(inputs["fc_b"], f32).copy(),
        w1t=w1t, fb1=np.asarray(inputs["ff_b1"], f32).copy(),
        w2t=w2t, fb2=np.asarray(inputs["ff_b2"], f32).copy(),
    )

    in_maps = []
    for i in range(8):
        b, half = i // 2, i % 2
        t0 = half * TQ
        noisyH = np.zeros((NHW, D), f32)
        lo, hi = t0 - P, t0 + 640
        clo, chi = max(lo, 0), min(hi, T)
        noisyH[clo - lo:chi - lo] = noisy[b, clo:chi]
        hm = np.zeros((NHW,), f32)
        hm[clo - lo:chi - lo] = 1.0
        maskb = np.where(np.arange(T) >= clean_len[b], NEG, 0.0).astype(f32)
        mod = np.stack([sh_msa[b], 1.0 + sc_msa[b], g_msa[b],
                        sh_mlp[b], 1.0 + sc_mlp[b], g_mlp[b]]).astype(f32)
        m = dict(common)
        m.update(noisyH=noisyH, clean=clean[b].copy(),
                 hm=np.broadcast_to(hm, (P, NHW)).astype(bf).copy(),
                 maskb=maskb, mod=mod)
        in_maps.append(m)

    global _LAST_INMAPS
    _LAST_INMAPS = in_maps
    res = run_bass_kernel_spmd(nc, in_maps, core_ids=list(range(8)))
    out = np.empty((B, T, D), f32)
    for i in range(8):
        b, half = i // 2, i % 2
        out[b, half * TQ:(half + 1) * TQ] = res.results[i]["out"]
    return out


_LAST_INMAPS = None


def run_profiled(tmpdir=None):
    """Re-run the last kernel invocation with NTFF tracing; return exec ns."""
    if _NC_CACHE is None or _LAST_INMAPS is None:
        return None
    res = run_bass_kernel_spmd(_NC_CACHE, _LAST_INMAPS,
                               core_ids=list(range(8)), trace=True,
                               tmpdir=tmpdir)
    return res.exec_time_ns


if __name__ == "__main__":
    build_nc()
    print("build ok")


# revision 38
# speedup vs baseline: 1.1402x; 1.1402x over previous
# Trainium2 Bass kernel for nn_CrossAttention_noise (B=4, T1=T2=1024, D=1024,
# H=16, DK=64, K=13, FF=4096), SPMD over 8 NeuronCores.
#
# Sharding: core i handles batch b=i//2 and query-token half t0=(i%2)*512.
# Each core computes its 512 output tokens end-to-end (the K/V convolution
# over the full clean sequence is duplicated between the two cores of a
# batch; no collectives).  Big matmuls run in bf16 with fp32 PSUM
# accumulation; layernorms / softmax / residuals stay fp32.
#
# Layout convention: "T" suffix = channels on partitions, tokens on the free
# dim (the natural matmul layout here); plain tiles = tokens on partitions.
import numpy as np
import ml_dtypes
from contextlib import ExitStack

import concourse.bass as bass
import concourse.mybir as mybir
import concourse.tile as tile
from concourse import bacc
from concourse.bass_utils import run_bass_kernel_spmd
from concourse.masks import make_identity

BF16 = mybir.dt.bfloat16
F32 = mybir.dt.float32
AF = mybir.ActivationFunctionType
ALU = mybir.AluOpType
AX = mybir.AxisListType

B, T, D, H, DK, KW, FF = 4, 1024, 1024, 16, 64, 13, 4096
TQ = 512          # query tokens per core
NHW = 768         # noisy halo window rows (zero-padded on host)
NT2W = 528        # nt2 width (valid cols 0..523)
CT2W = 1040       # ct2 width (valid cols 0..1035)
P = 128
EPS1, EPS2 = 1e-5, 1e-6
NEG = -1.0e30


def _ln_apply(nc, pool, x, out, eps_ap, act_apply=False):
    """out = (x - mean)/sqrt(var + eps) rowwise; x [p, D] f32 in SBUF.

    Heavy passes run on the scalar engine (accum_out reductions + fused
    scale/bias apply); DVE only does tiny [p,1] ops."""
    p = x.shape[0]
    s = pool.tile([P, 1], F32, tag="ln_s", name="ln_s")[:p]
    sq = pool.tile([P, 1], F32, tag="ln_sq", name="ln_sq")[:p]
    scr = pool.tile([P, D], BF16, tag="ln_scr", name="ln_scr", bufs=2)[:p]
    nc.vector.reduce_sum(s, x, axis=AX.X)
    nc.scalar.activation(scr, x, AF.Square, accum_out=sq)
    mu = pool.tile([P, 1], F32, tag="ln_mu", name="ln_mu")[:p]
    nc.vector.tensor_scalar_mul(mu, s, 1.0 / D)
    musq = pool.tile([P, 1], F32, tag="ln_musq", name="ln_musq")[:p]
    nc.vector.tensor_tensor(musq, mu, mu, ALU.mult)
    var = pool.tile([P, 1], F32, tag="ln_var", name="ln_var")[:p]
    nc.vector.tensor_scalar(var, sq, 1.0 / D, musq, ALU.mult, ALU.subtract)
    std = pool.tile([P, 1], F32, tag="ln_std", name="ln_std")[:p]
    nc.scalar.activation(std, var, AF.Sqrt, bias=eps_ap[:p])
    rstd = pool.tile([P, 1], F32, tag="ln_rstd", name="ln_rstd")[:p]
    nc.vector.reciprocal(rstd, std)
    beta = pool.tile([P, 1], F32, tag="ln_beta", name="ln_beta")[:p]
    nc.vector.tensor_tensor(beta, mu, rstd, ALU.mult)
    nc.vector.tensor_scalar_mul(beta, beta, -1.0)
    if act_apply:
        nc.scalar.activation(out, x, AF.Identity, bias=beta, scale=rstd)
    else:
        nc.vector.tensor_scalar(out, x, rstd, beta, ALU.mult, ALU.add)


def _nt2_rng(r, shift, width):
    """Dest/src col ranges for copying transpose block r into a shifted row."""
    lo = r * P - shift
    hi = lo + P
    d0, d1 = max(lo, 0), min(hi, width)
    if d1 <= d0:
        return None
    return d0, d1, d0 - lo


def build_nc():
    nc = bacc.Bacc("TRN2", target_bir_lowering=False, debug=False,
                   num_devices=8)
    dt = {}

    def din(name, shape, dtype):
        dt[name] = nc.dram_tensor(name, list(shape), dtype,
                                  kind="ExternalInput").ap()

    din("noisyH", (NHW, D), F32)          # rows [t0-128, t0+640), zero padded
    din("clean", (T, D), F32)
    din("hm", (P, NHW), BF16)              # halo-token validity (rows equal)
    din("maskb", (T,), F32)               # 0 / -1e30 additive key mask
    din("mod", (6, D), F32)               # sh_msa,1+sc_msa,g_msa,sh_mlp,1+sc_mlp,g_mlp
    din("lng", (P, D), F32)               # ln_noisy_g broadcast to 128 rows
    din("lnb", (P, D), F32)
    din("clng", (D,), F32)
    din("clnb", (D,), F32)
    din("wql", (P, H, 7, DK), BF16)
    din("wkl", (P, H, 7, DK), BF16)
    din("wvl", (P, H, 7, DK), BF16)
    din("bq", (D,), F32)
    din("bk", (D,), F32)
    din("bv", (D,), F32)
    din("fcw", (8, P, 8, P), BF16)        # fc_w.T tiles [mc][kp][ko][mj]
    din("fcb", (D,), F32)
    din("w1t", (32, P, 8, P), BF16)       # ff_w1.T tiles [mc][kp][ko][mj]
    din("fb1", (FF,), F32)
    din("w2t", (8, 4, P, 8, P), BF16)     # ff_w2.T tiles [mc][kq][kp][k8][mj]
    din("fb2", (D,), F32)
    out_ap = nc.dram_tensor("out", [TQ, D], F32, kind="ExternalOutput").ap()

    with tile.TileContext(nc) as tc:
        _emit(tc, dt, out_ap)
    nc.compile()
    return nc


def _emit(tc, dt, out_ap):
    nc = tc.nc
    with ExitStack() as ctx:
        const = ctx.enter_context(tc.tile_pool(name="const", bufs=1))
        small = ctx.enter_context(tc.tile_pool(name="small", bufs=3))
        lnio = ctx.enter_context(tc.tile_pool(name="lnio", bufs=3))
        big = ctx.enter_context(tc.tile_pool(name="bigsb", bufs=1))
        trans = ctx.enter_context(tc.tile_pool(name="trans", bufs=3))
        wpool = ctx.enter_context(tc.tile_pool(name="wstream", bufs=6))
        psc = ctx.enter_context(tc.tile_pool(name="psc", bufs=2, space="PSUM"))
        ppv = ctx.enter_context(tc.tile_pool(name="ppv", bufs=1, space="PSUM"))
        ptp = ctx.enter_context(tc.tile_pool(name="ptp", bufs=3, space="PSUM"))
        psm = ctx.enter_context(tc.tile_pool(name="psm", bufs=2, space="PSUM"))

        ident = const.tile([P, P], BF16)
        make_identity(nc, ident)
        eps1_t = const.tile([P, 1], F32)
        nc.vector.memset(eps1_t, EPS1)
        eps2_t = const.tile([P, 1], F32)
        nc.vector.memset(eps2_t, EPS2)

        def chanvec(name, w=8):
            t = const.tile([P, w], F32, tag=f"cv_{name}")
            nc.sync.dma_start(t, dt[name].rearrange("(m p) -> p m", p=P))
            return t

        bq_s, bk_s, bv_s = chanvec("bq"), chanvec("bk"), chanvec("bv")
        fcb_s, fb2_s = chanvec("fcb"), chanvec("fb2")
        clng_s, clnb_s = chanvec("clng"), chanvec("clnb")
        maskb_s = chanvec("maskb")
        fb1_s = chanvec("fb1", 32)
        mod_s = const.tile([P, 6, 8], F32)
        for s in range(6):
            nc.sync.dma_start(mod_s[:, s, :],
                              dt["mod"][s].rearrange("(m p) -> p m", p=P))
        sh_msa, sc_msa, g_msa = mod_s[:, 0, :], mod_s[:, 1, :], mod_s[:, 2, :]
        sh_mlp, sc_mlp, g_mlp = mod_s[:, 3, :], mod_s[:, 4, :], mod_s[:, 5, :]
        hm_s = const.tile([P, NHW], BF16)
        nc.sync.dma_start(hm_s, dt["hm"])
        lng_s = const.tile([P, D], F32)
        nc.sync.dma_start(lng_s, dt["lng"])
        lnb_s = const.tile([P, D], F32)
        nc.sync.dma_start(lnb_s, dt["lnb"])

        xres = big.tile([P, 4, D], F32)        # LN1 rows [t0, t0+512); later x
        attnT = big.tile([P, 8, TQ], BF16)     # concat_h(out_h/l_h), chan-major

        with tc.tile_pool(name="bigc", bufs=1) as bigc:
            # ---- Phase A: noisy LNs -> nt2 builds -> all q convs ------------
            lnpN_cm = tc.tile_pool(name="lnpN", bufs=1)
            lnpN = lnpN_cm.__enter__()
            lnall = [lnpN.tile([P, D], BF16, name=f"lnall_{i}")
                     for i in range(6)]  # noisy ln2 tiles
            for r in range(6):
                xt = lnio.tile([P, D], F32, tag="ln_in", bufs=2)
                nc.sync.dma_start(xt, dt["noisyH"][r * P:(r + 1) * P, :])
                if 1 <= r <= 4:
                    ln1 = xres[:, r - 1, :]
                else:
                    ln1 = lnio.tile([P, D], F32, tag="ln1_tmp", bufs=1)
                _ln_apply(nc, small, xt, ln1, eps1_t)
                nc.vector.tensor_tensor(ln1, ln1, lng_s, ALU.mult)
                nc.vector.tensor_tensor(ln1, ln1, lnb_s, ALU.add)
                _ln_apply(nc, small, ln1, lnall[r], eps2_t)

            nt2s, ct2s = [], []
            cp_eng = [nc.vector, nc.gpsimd]
            for m in range(8):
                nt2m = bigc.tile([P, 2, NT2W], BF16, name=f"nt2_{m}")
                nt2s.append(nt2m)
                tmn = trans.tile([P, NHW], BF16, tag="tmn", bufs=2)
                for r in range(6):
                    pt = ptp.tile([P, P], BF16, tag="tpbf")
                    nc.tensor.transpose(pt, lnall[r][:, m * P:(m + 1) * P],
                                        ident)
                    dst = tmn[:, r * P:(r + 1) * P]
                    if r % 2 == 0:
                        nc.vector.tensor_scalar(dst, pt, sc_msa[:, m:m + 1],
                                                sh_msa[:, m:m + 1],
                                                ALU.mult, ALU.add)
                    else:
                        nc.scalar.activation(dst, pt, AF.Identity,
                                             bias=sh_msa[:, m:m + 1],
                                             scale=sc_msa[:, m:m + 1])
                for hh in range(2):
                    sl = slice(hh * DK, (hh + 1) * DK)
                    e0, e1 = cp_eng[hh], cp_eng[1 - hh]
                    e0.tensor_tensor(nt2m[0:DK, hh, 0:524], tmn[sl, 122:646],
                                     hm_s[sl, 122:646], ALU.mult)
                    e1.tensor_tensor(nt2m[DK:P, hh, 0:524], tmn[sl, 123:647],
                                     hm_s[sl, 123:647], ALU.mult)

            lnpN_cm.__exit__(None, None, None)
            hpool_cm = tc.tile_pool(name="hpool", bufs=2)
            hpool = hpool_cm.__enter__()

            def conv(h, wname, bias_s, x2, nchunk, name, bufs=2):
                hp, hc = h % 2, h // 2
                wsb = wpool.tile([P, 7, DK], BF16, tag="convw", bufs=4,
                                 name=f"w_{name}")
                nc.sync.dma_start(wsb, dt[wname][:, h])
                outT = hpool.tile([DK, nchunk * TQ], BF16, tag=f"cv_{name}",
                                  bufs=bufs, name=f"cv_{name}_{h}")
                for c in range(nchunk):
                    ps = psm.tile([DK, TQ], F32, tag="conv")
                    for j in range(7):
                        nc.tensor.matmul(
                            ps, wsb[:, j, :],
                            x2[:, hp, c * TQ + 2 * j:c * TQ + 2 * j + TQ],
                            start=(j == 0), stop=(j == 6))
                    if (h + c) % 2 == 0:
                        nc.vector.tensor_scalar_add(
                            outT[:, c * TQ:(c + 1) * TQ], ps,
                            bias_s[hp * DK:(hp + 1) * DK, hc:hc + 1])
                    else:
                        nc.scalar.activation(
                            outT[:, c * TQ:(c + 1) * TQ], ps, AF.Identity,
                            bias=bias_s[hp * DK:(hp + 1) * DK, hc:hc + 1])
                return outT

            qTs = [conv(h, "wql", bq_s, nt2s[h // 2], 1, f"q{h}", bufs=1)
                   for h in range(H)]

            # ---- Phase B: clean LNs -> ct2 builds ---------------------------
            lnpC_cm = tc.tile_pool(name="lnpC", bufs=1)
            lnpC = lnpC_cm.__enter__()
            clnall = [lnpC.tile([P, D], BF16, name=f"clnall_{i}")
                      for i in range(8)]
            for r in range(8):
                xt = lnio.tile([P, D], F32, tag="ln_in", bufs=2)
                nc.sync.dma_start(xt, dt["clean"][r * P:(r + 1) * P, :])
                _ln_apply(nc, small, xt, clnall[r], eps1_t)
            for m in range(8):
                ct2m = bigc.tile([P, 2, CT2W], BF16, name=f"ct2_{m}")
                ct2s.append(ct2m)
                for hh in range(2):
                    nc.gpsimd.memset(ct2m[0:DK, hh, 0:6], 0.0)
                    nc.gpsimd.memset(ct2m[0:DK, hh, 1030:CT2W], 0.0)
                    nc.gpsimd.memset(ct2m[DK:P, hh, 0:5], 0.0)
                    nc.gpsimd.memset(ct2m[DK:P, hh, 1029:CT2W], 0.0)
                tmc = trans.tile([P, T], BF16, tag="tmc", bufs=2)
                for r in range(8):
                    pt = ptp.tile([P, P], BF16, tag="tpbf")
                    nc.tensor.transpose(pt, clnall[r][:, m * P:(m + 1) * P],
                                        ident)
                    dst = tmc[:, r * P:(r + 1) * P]
                    if r % 2 == 0:
                        nc.vector.tensor_scalar(dst, pt, clng_s[:, m:m + 1],
                                                clnb_s[:, m:m + 1],
                                                ALU.mult, ALU.add)
                    else:
                        nc.scalar.activation(dst, pt, AF.Identity,
                                             bias=clnb_s[:, m:m + 1],
                                             scale=clng_s[:, m:m + 1])
                for hh in range(2):
                    sl = slice(hh * DK, (hh + 1) * DK)
                    e0, e1 = cp_eng[hh], cp_eng[1 - hh]
                    e0.tensor_copy(ct2m[0:DK, hh, 6:1030], tmc[sl, :])
                    e1.tensor_copy(ct2m[DK:P, hh, 5:1029], tmc[sl, :])
            lnpC_cm.__exit__(None, None, None)

            # ---- Phase C: per-head conv K/V + cross attention ---------------
            for h in range(H):
                hp = h % 2
                hc = h // 2
                ct2 = ct2s[hc]
                kT = conv(h, "wkl", bk_s, ct2, 2, "k")
                vT = conv(h, "wvl", bv_s, ct2, 2, "v")
                qT = qTs[h]

                # v65: v tokens-on-partitions plus ones column for row sums
                v65 = hpool.tile([P, 8, 66], BF16, tag="v65", bufs=1)
                nc.vector.memset(v65[:, :, 64:65], 1.0)
                for c in range(8):
                    pt = ptp.tile([P, P], BF16, tag="tpbf")
                    nc.tensor.transpose(pt[:, :DK], vT[:, c * P:(c + 1) * P],
                                        ident[:DK, :DK])
                    nc.vector.tensor_copy(v65[:, c, 0:DK], pt[:, :DK])

                # transposed scores; fused mask/scale/exp (T2 on partitions)
                pT = hpool.tile([P, 8, TQ], BF16, tag="pT", bufs=2)
                for c in range(8):
                    ps = psc.tile([P, TQ], F32, tag="sc")
                    nc.tensor.matmul(ps, kT[:, c * P:(c + 1) * P], qT,
                                     start=True, stop=True)
                    nc.scalar.activation(pT[:, c, :], ps, AF.Exp,
                                         bias=maskb_s[:, c:c + 1], scale=0.125)

                # PV: out[65, TQ] = [v|1]^T @ p (row 64 = softmax denominator)
                pv = ppv.tile([P, TQ], F32, tag="pv")
                for c in range(8):
                    nc.tensor.matmul(pv[:65, :], v65[:, c, 0:65], pT[:, c, :],
                                     start=(c == 0), stop=(c == 7))
                linv = trans.tile([1, TQ], F32, tag="linv")
                nc.vector.reciprocal(linv, pv[64:65, :])
                bc_sb = trans.tile([DK, TQ], F32, tag="bcsb", bufs=2)
                nc.gpsimd.partition_broadcast(bc_sb, linv)
                nc.vector.tensor_tensor(attnT[hp * DK:(hp + 1) * DK, hc, :],
                                        pv[0:DK, :], bc_sb, ALU.mult)
            hpool_cm.__exit__(None, None, None)

        # ---- Phase D: fc projection + gate + residual into xres -------------
        fcgs = []
        for m in range(8):
            wt = wpool.tile([P, 8, P], BF16, tag="wt")
            nc.sync.dma_start(wt, dt["fcw"][m])
            ps = psc.tile([P, TQ], F32, tag="sc")
            for k in range(8):
                nc.tensor.matmul(ps, wt[:, k, :], attnT[:, k, :],
                                 start=(k == 0), stop=(k == 7))
            fcg = trans.tile([P, TQ], BF16, tag="fcg", bufs=8,
                             name=f"fcg_{m}")
            nc.vector.tensor_scalar(fcg, ps, fcb_s[:, m:m + 1],
                                    g_msa[:, m:m + 1], ALU.add, ALU.mult)
            fcgs.append(fcg)
        for j in range(4):
            for m in range(8):
                pt = ptp.tile([P, P], BF16, tag="tpbf")
                nc.tensor.transpose(pt, fcgs[m][:, j * P:(j + 1) * P], ident)
                nc.vector.tensor_tensor(xres[:, j, m * P:(m + 1) * P], pt,
                                        xres[:, j, m * P:(m + 1) * P], ALU.add)

        # ---- Phase E: LN3 + mlp modulation -> n2T ---------------------------
        bigf_cm = tc.tile_pool(name="bigf", bufs=1)
        bigf = bigf_cm.__enter__()
        n2T = bigf.tile([P, 8, TQ], BF16)
        for s in range(4):
            l3 = lnio.tile([P, D], BF16, tag="ln2b")
            _ln_apply(nc, small, xres[:, s, :], l3, eps2_t)
            for m in range(8):
                pt = ptp.tile([P, P], BF16, tag="tpbf")
                nc.tensor.transpose(pt, l3[:, m * P:(m + 1) * P], ident)
                nc.vector.tensor_scalar(n2T[:, m, s * P:(s + 1) * P], pt,
                                        sc_mlp[:, m:m + 1], sh_mlp[:, m:m + 1],
                                        ALU.mult, ALU.add)

        # ---- Phase F: FFN (single pass; SBUF freed by bigc/hpool exit) ------
        if True:
            ffa = bigf.tile([P, 32, TQ], BF16)
            for m in range(32):
                wt = wpool.tile([P, 8, P], BF16, tag="wt")
                nc.sync.dma_start(wt, dt["w1t"][m])
                ps = psc.tile([P, TQ], F32, tag="sc")
                for k in range(8):
                    nc.tensor.matmul(ps, wt[:, k, :], n2T[:, k, :],
                                     start=(k == 0), stop=(k == 7))
                nc.scalar.activation(ffa[:, m, :], ps, AF.Gelu_apprx_tanh,
                                     bias=fb1_s[:, m:m + 1])
            for m in range(8):
                ps = psc.tile([P, TQ], F32, tag="sc")
                for kq in range(4):
                    wt = wpool.tile([P, 8, P], BF16, tag="wt")
                    nc.sync.dma_start(wt, dt["w2t"][m, kq])
                    for k8 in range(8):
                        k = kq * 8 + k8
                        nc.tensor.matmul(ps, wt[:, k8, :], ffa[:, k, :],
                                         start=(k == 0), stop=(k == 31))
                ffog = trans.tile([P, TQ], BF16, tag="ffog", bufs=2)
                nc.vector.tensor_scalar(ffog, ps, fb2_s[:, m:m + 1],
                                        g_mlp[:, m:m + 1], ALU.add, ALU.mult)
                for j in range(4):
                    pt = ptp.tile([P, P], BF16, tag="tpbf")
                    nc.tensor.transpose(pt, ffog[:, j * P:(j + 1) * P], ident)
                    nc.vector.tensor_tensor(xres[:, j, m * P:(m + 1) * P], pt,
                                            xres[:, j, m * P:(m + 1) * P],
                                            ALU.add)
        bigf_cm.__exit__(None, None, None)

        for s in range(4):
            nc.sync.dma_start(out_ap[s * P:(s + 1) * P, :], xres[:, s, :])


# --------------------------- host side --------------------------------------
_NC_CACHE = None


def _prep_conv_w(w):
    # w: (D, DK, KW) grouped conv weights -> [128, H, 7, DK] bf16 tap-pair lhsT
    wr = w.reshape(H, DK, DK, KW)                      # [h, m, c, tap]
    arr = np.zeros((P, H, 7, DK), np.float32)
    arr[0:DK] = wr[:, :, :, 0::2].transpose(2, 0, 3, 1)      # taps 0,2,..,12
    arr[DK:P, :, 0:6] = wr[:, :, :, 1::2].transpose(2, 0, 3, 1)
    return arr.astype(ml_dtypes.bfloat16)


def kernel(**inputs):
    global _NC_CACHE
    if _NC_CACHE is None:
        _NC_CACHE = build_nc()
    nc = _NC_CACHE

    f32 = np.float32
    bf = ml_dtypes.bfloat16
    noisy = np.asarray(inputs["noisy_feats"], f32)
    clean = np.asarray(inputs["clean_feats"], f32)
    t = np.asarray(inputs["t"], f32)
    clean_len = np.asarray(inputs["clean_lengths"]).astype(np.int64)

    # AdaLayerNormZero on host (0.02% of FLOPs): emb = silu(t) @ ada_w.T + b
    st = t * (1.0 / (1.0 + np.exp(-t, dtype=f32)))
    emb = st @ np.asarray(inputs["ada_w"], f32).T + np.asarray(inputs["ada_b"], f32)
    sh_msa, sc_msa, g_msa, sh_mlp, sc_mlp, g_mlp = np.split(emb, 6, axis=1)

    wql = _prep_conv_w(np.asarray(inputs["wq"], f32))
    wkl = _prep_conv_w(np.asarray(inputs["wk"], f32))
    wvl = _prep_conv_w(np.asarray(inputs["wv"], f32))
    fcw = np.asarray(inputs["fc_w"], f32).T.reshape(8, P, 8, P) \
        .transpose(2, 1, 0, 3).astype(bf).copy()
    w1t = np.asarray(inputs["ff_w1"], f32).T.reshape(8, P, 32, P) \
        .transpose(2, 1, 0, 3).astype(bf).copy()
    w2t = np.asarray(inputs["ff_w2"], f32).T.reshape(32, P, 8, P) \
        .transpose(2, 0, 1, 3).reshape(8, 4, 8, P, P) \
        .transpose(0, 1, 3, 2, 4).astype(bf).copy()

    common = dict(
        lng=np.broadcast_to(np.asarray(inputs["ln_noisy_g"], f32), (P, D)).copy(),
        lnb=np.broadcast_to(np.asarray(inputs["ln_noisy_b"], f32), (P, D)).copy(),
        clng=np.asarray(inputs["ln_clean_g"], f32).copy(),
        clnb=np.asarray(inputs["ln_clean_b"], f32).copy(),
        wql=wql, wkl=wkl, wvl=wvl,
        bq=np.asarray(inputs["bq"], f32).copy(),
        bk=np.asarray(inputs["bk"], f32).copy(),
        bv=np.asarray(inputs["bv"], f32).copy(),
        fcw=fcw, fcb=np.asarray(inputs["fc_b"], f32).copy(),
        w1t=w1t, fb1=np.asarray(inputs["ff_b1"], f32).copy(),
        w2t=w2t, fb2=np.asarray(inputs["ff_b2"], f32).copy(),
    )

    in_maps = []
    for i in range(8):
        b, half = i // 2, i % 2
        t0 = half * TQ
        noisyH = np.zeros((NHW, D), f32)
        lo, hi = t0 - P, t0 + 640
        clo, chi = max(lo, 0), min(hi, T)
        noisyH[clo - lo:chi - lo] = noisy[b, clo:chi]
        hm = np.zeros((NHW,), f32)
        hm[clo - lo:chi - lo] = 1.0
        maskb = np.where(np.arange(T) >= clean_len[b], NEG, 0.0).astype(f32)
        mod = np.stack([sh_msa[b], 1.0 + sc_msa[b], g_msa[b],
                        sh_mlp[b], 1.0 + sc_mlp[b], g_mlp[b]]).astype(f32)
        m = dict(common)
        m.update(noisyH=noisyH, clean=clean[b].copy(),
                 hm=np.broadcast_to(hm, (P, NHW)).astype(bf).copy(),
                 maskb=maskb, mod=mod)
        in_maps.append(m)

    global _LAST_INMAPS
    _LAST_INMAPS = in_maps
    res = run_bass_kernel_spmd(nc, in_maps, core_ids=list(range(8)))
    out = np.empty((B, T, D), f32)
    for i in range(8):
        b, half = i // 2, i % 2
        out[b, half * TQ:(half + 1) * TQ] = res.results[i]["out"]
    return out


_LAST_INMAPS = None


def run_profiled(tmpdir=None):
    """Re-run the last kernel invocation with NTFF tracing; return exec ns."""
    if _NC_CACHE is None or _LAST_INMAPS is None:
        return None
    res = run_bass_kernel_spmd(_NC_CACHE, _LAST_INMAPS,
                               core_ids=list(range(8)), trace=True,
                               tmpdir=tmpdir)
    return res.exec_time_ns


if __name__ == "__main__":
    build_nc()
    print("build ok")
